# revision 30
# baseline (speedup 1.0000x reference)
"""Trainium2 Bass kernel for nn_JSDPosLoss: JSD loss over top-k retrieved rows.

Contract: kernel(**inputs) takes FULL numpy inputs, returns FULL output (f32 scalar).
Data-parallel over batch across 8 NeuronCores (4 batches/core).

Host prep (sharding): gathers sample_z / sample_z_dis (rand_idx is a host-known
input), transposes z_pos to (b, d, hw) so the device streams contraction-major
tiles directly, builds the JSD P matrix (pure broadcast of sample_z_dis).

Per-core device work:
  - stream z_posT tiles (16 MiB, the memory-bound part) across 3 DMA queues
  - matmul vs pre-gathered sample_z^T -> attn rows (batch bi at partitions
    32*bi..32*bi+2; compute-engine SBUF access must start at 0/32/64/96)
  - top-10 per (batch, query) row via DVE max8/max_index/match_replace
  - indirect-DMA gather of selected z_pos_dis rows (120 rows x 512)
  - JSD elementwise terms + free-dim reduction -> (120, 1) partial sums
Host: final scalar reduce + scale.
"""

import numpy as np

import concourse.bass as bass
import concourse.bacc as bacc
import concourse.mybir as mybir
import concourse.tile as tile
from concourse.bass_utils import run_bass_kernel_spmd

# Problem dims (hardcoded per contract)
B, H, W, D, NPQ = 32, 64, 64, 256, 512
HW = H * W                  # 4096
NQ, NPOS = 3, 10
NCORES = 8
BPC = B // NCORES           # 4 batches per core
NROW = BPC * NQ             # 12 attention rows per core
NPAIR = BPC * NQ * NPOS     # 120 JSD pair-rows per core

F32 = mybir.dt.float32
F32R = mybir.dt.float32r
U32 = mybir.dt.uint32

JH = 2048                   # j-columns per load (1 MiB per DMA)
MM_DTYPE = F32R            # matmul input dtype (F32 exact / F32R relaxed)


def build_kernel():
    nc = bacc.Bacc("TRN2", target_bir_lowering=False, debug=False,
                   num_devices=NCORES)

    # z_pos transposed on host: (BPC, 2, 128, HW), [bi, ck, cl, j]
    zpt = nc.dram_tensor("zpt", [BPC, 2, 128, HW], MM_DTYPE,
                         kind="ExternalInput").ap()
    zpdis = nc.dram_tensor("zpdis", [BPC * HW, NPQ], F32,
                           kind="ExternalInput").ap()
    szt = nc.dram_tensor("szt", [2, 128, 128], MM_DTYPE,
                         kind="ExternalInput").ap()
    pmat = nc.dram_tensor("pmat", [NPAIR, NPQ], F32, kind="ExternalInput").ap()
    boffs = nc.dram_tensor("boffs", [128, 1], F32, kind="ExternalInput").ap()
    out = nc.dram_tensor("out", [NPAIR, 1], F32, kind="ExternalOutput").ap()

    with tile.TileContext(nc) as tc:
        _body(tc, nc, zpt, zpdis, szt, pmat, boffs, out)
    nc.compile()
    return nc


def _body(tc, nc, zpt, zpdis, szt, pmat, boffs, out):
    NJQ = 4                     # j-quarters; topk rounds stream per quarter
    JQ = HW // NJQ              # 1024
    with (
        tc.tile_pool(name="const", bufs=1) as cpool,
        tc.tile_pool(name="load", bufs=6) as lpool,
        tc.tile_pool(name="atp", bufs=4, space="PSUM") as atp_pool,
        tc.tile_pool(name="qtk", bufs=2) as qpool,
        tc.tile_pool(name="small", bufs=1) as spool,
        tc.tile_pool(name="jsd", bufs=1) as jpool,
    ):
        # sample_z^T padded to 32 columns per batch (queries at 32*bi+q,
        # zeros elsewhere): matmuls then initialize all 128 attn partitions
        # (MM_DTYPE tiles: the DMA rounds f32 -> f32r at the producer, as the
        # BIR verifier requires for fp32r matmul operands)
        szt_sb = cpool.tile([128, 256], MM_DTYPE)
        nc.sync.dma_start(szt_sb[:, 0:128], szt[0])
        nc.sync.dma_start(szt_sb[:, 128:256], szt[1])

        # constants / independent loads, issued early
        boff = spool.tile([128, 1], F32)
        nc.sync.dma_start(boff[:], boffs[:, :])
        pm = jpool.tile([NPAIR, NPQ], F32)
        nc.scalar.dma_start(pm[:], pmat[:, :])

        # attention rows in SBUF: batch bi at partitions 32*bi..32*bi+2
        # (fp32r matmuls may only write PSUM at partition base 0, so each
        # (3, 512) slice lands in a partition-0 PSUM tile and DVE moves it)
        attn = cpool.tile([128, HW], F32)

        # per-quarter candidate maxima (top-16 per quarter per row)
        cand = cpool.tile([128, NJQ * 16], F32)

        # DMA issue queues: SP + ACT (HWDGE) + Pool (SWDGE), round-robin
        dma_engines = [nc.sync, nc.gpsimd, nc.scalar, nc.sync, nc.gpsimd]
        qi = 0

        for jq in range(NJQ):
            for bi in range(BPC):
                lds = []
                for ck in range(2):
                    ld = lpool.tile([128, JQ], MM_DTYPE, tag=f"ld{ck}")
                    eng = dma_engines[qi % len(dma_engines)]
                    qi += 1
                    eng.dma_start(ld[:], zpt[bi, ck, :, jq * JQ:(jq + 1) * JQ])
                    lds.append(ld)
                at_ps = atp_pool.tile([32, JQ], F32, tag="at_ps")
                for js in range(JQ // 512):
                    for ck in range(2):
                        nc.tensor.matmul(
                            at_ps[:, js * 512:(js + 1) * 512],
                            lhsT=szt_sb[:, ck * 128 + 32 * bi:
                                        ck * 128 + 32 * bi + 32],
                            rhs=lds[ck][:, js * 512:(js + 1) * 512],
                            start=(ck == 0), stop=(ck == 1))
                # one ACT copy per (batch, quarter); M=32 with zero-padded
                # queries also initializes the garbage attn partitions
                nc.scalar.copy(
                    attn[32 * bi:32 * bi + 32, jq * JQ:(jq + 1) * JQ],
                    at_ps[:])
            # streamed topk round for this quarter: top-16 values per row
            aq = attn[:, jq * JQ:(jq + 1) * JQ]
            c0 = cand[:, jq * 16:jq * 16 + 8]
            c1 = cand[:, jq * 16 + 8:jq * 16 + 16]
            nc.vector.max(c0, aq)
            tmpq = qpool.tile([128, JQ], F32, tag="tmpq")
            nc.vector.match_replace(tmpq[:], in_to_replace=c0,
                                    in_values=aq, imm_value=-1e30)
            nc.vector.max(c1, tmpq[:])

        # ---- merge quarters: top-10 values per row out of 64 candidates ----
        mv1 = spool.tile([128, 8], F32)
        nc.vector.max(mv1[:], cand[:])
        cand2 = spool.tile([128, NJQ * 16], F32)
        nc.vector.match_replace(cand2[:], in_to_replace=mv1[:],
                                in_values=cand[:], imm_value=-1e30)
        mv2 = spool.tile([128, 8], F32)
        nc.vector.max(mv2[:], cand2[:])
        mv10 = spool.tile([128, NPOS], F32)
        nc.vector.tensor_copy(mv10[:, 0:8], mv1[:])
        nc.vector.tensor_copy(mv10[:, 8:NPOS], mv2[:, 0:2])

        # ---- resolve indices: two max_index scans of the full attn row ----
        ix1 = spool.tile([128, 8], U32)
        ix2 = spool.tile([128, 8], U32)
        nc.vector.max_index(ix1[:], mv10[:, 0:8], attn[:])
        nc.vector.max_index(ix2[:], mv10[:, 2:NPOS], attn[:])

        idx10 = spool.tile([128, NPOS], U32)
        nc.vector.tensor_copy(idx10[:, 0:8], ix1[:])
        nc.vector.tensor_copy(idx10[:, 8:NPOS], ix2[:, 6:8])

        # add bi*HW so indices address the flattened (BPC*HW, NPQ) table
        # (f32 arithmetic: indices < 16384 are exact; cast back to u32 after)
        idx10f = spool.tile([128, NPOS], F32)
        nc.vector.tensor_copy(idx10f[:], idx10[:])
        nc.vector.tensor_scalar(idx10f[:], idx10f[:], boff[:], None,
                                op0=mybir.AluOpType.add)
        nc.vector.tensor_copy(idx10[:], idx10f[:])

        # flatten the 12 valid rows -> (120, 1); order (bi, q, k)
        # (spread across queues so the tiny DMAs overlap)
        idx_flat = spool.tile([NPAIR, 1], U32)
        for bi, eng in zip(range(BPC),
                           (nc.sync, nc.scalar, nc.gpsimd, nc.sync)):
            eng.dma_start(idx_flat[30 * bi:30 * (bi + 1), :],
                          idx10[32 * bi:32 * bi + NQ, :])

        # ---- gather the selected z_pos_dis rows ----
        gmat = jpool.tile([NPAIR, NPQ], F32)
        nc.gpsimd.indirect_dma_start(
            out=gmat[:], out_offset=None,
            in_=zpdis[:, :],
            in_offset=bass.IndirectOffsetOnAxis(ap=idx_flat[:, :1], axis=0))

        # ---- JSD terms: xlogy(p,p) + xlogy(g,g) - (p+g)*log(clip((p+g)/2)) ----
        # Ln(x*scale + bias) fusion on ACT: bias 1e-7/1e-38 stands in for the
        # clip/xlogy(0,0) guards (error <= ~1e-6 relative, values in [0, 1))
        bias7 = jpool.tile([NPAIR, 1], F32)
        nc.vector.memset(bias7[:], 1e-7)
        bias38 = jpool.tile([NPAIR, 1], F32)
        nc.vector.memset(bias38[:], 1e-38)

        s = jpool.tile([NPAIR, NPQ], F32)
        nc.vector.tensor_add(s[:], pm[:], gmat[:])
        m = jpool.tile([NPAIR, NPQ], F32)
        nc.scalar.activation(m[:], s[:], mybir.ActivationFunctionType.Ln,
                             bias=bias7[:], scale=0.5)

        xp = jpool.tile([NPAIR, NPQ], F32)
        nc.scalar.activation(xp[:], pm[:], mybir.ActivationFunctionType.Ln,
                             bias=bias38[:])
        nc.vector.tensor_mul(xp[:], xp[:], pm[:])

        xg = jpool.tile([NPAIR, NPQ], F32)
        nc.scalar.activation(xg[:], gmat[:], mybir.ActivationFunctionType.Ln,
                             bias=bias38[:])
        nc.vector.tensor_mul(xg[:], xg[:], gmat[:])

        nc.vector.tensor_mul(s[:], s[:], m[:])     # s = (p+g) * m
        nc.vector.tensor_add(xp[:], xp[:], xg[:])
        nc.vector.tensor_sub(xp[:], xp[:], s[:])

        red = jpool.tile([NPAIR, 1], F32)
        nc.vector.tensor_reduce(red[:], xp[:], axis=mybir.AxisListType.X,
                                op=mybir.AluOpType.add)
        nc.sync.dma_start(out[:, :], red[:])


_CACHE = {}


def _prep_in_maps(z, z_pos, z_dis, z_pos_dis, rand_idx):
    zf = z.reshape(B, HW, D)
    zpdf = z_pos_dis.reshape(B, HW, NPQ).astype(np.float32, copy=False)
    zposf = z_pos.reshape(B, HW, D).astype(np.float32, copy=False)
    zdf = z_dis.reshape(B, HW, NPQ)

    ridx = rand_idx.astype(np.int64)
    sample_z = np.take_along_axis(zf, ridx[..., None], axis=1)       # (B,3,D)
    sample_z_dis = np.take_along_axis(zdf, ridx[..., None], axis=1)  # (B,3,NPQ)

    in_maps = []
    for c in range(NCORES):
        bs = slice(c * BPC, (c + 1) * BPC)
        # zpt[bi, ck, cl, j] = z_pos[4c+bi, j, 128*ck+cl]
        zpt = np.ascontiguousarray(
            zposf[bs].transpose(0, 2, 1).reshape(BPC, 2, 128, HW))
        # szt[ck, cl, 32*bi+q] = sample_z[4c+bi, q, 128*ck+cl], zero-pad
        sz = sample_z[bs]                                  # (BPC, 3, D)
        szt = np.zeros((2, 128, 128), np.float32)
        szt_q = sz.reshape(BPC * NQ, 2, 128).transpose(1, 2, 0)  # (2,128,12)
        for bi in range(BPC):
            szt[:, :, 32 * bi:32 * bi + NQ] = szt_q[:, :, NQ * bi:NQ * bi + NQ]
        # pmat row 30*bi + i = sample_z_dis[4c+bi, i % 3]
        szd = sample_z_dis[bs]                             # (BPC, 3, NPQ)
        i = np.arange(NQ * NPOS)
        pmatc = np.ascontiguousarray(
            szd[:, i % NQ, :].reshape(NPAIR, NPQ)).astype(np.float32)
        boffs = np.zeros((128, 1), np.float32)
        for bi in range(BPC):
            boffs[32 * bi:32 * bi + NQ, 0] = bi * HW
        in_maps.append({
            "zpt": zpt,
            "zpdis": np.ascontiguousarray(zpdf[bs].reshape(BPC * HW, NPQ)),
            "szt": szt,
            "pmat": pmatc,
            "boffs": boffs,
        })
    return in_maps


def kernel(z, z_pos, z_dis, z_pos_dis, rand_idx):
    if "nc" not in _CACHE:
        _CACHE["nc"] = build_kernel()
    nc = _CACHE["nc"]
    in_maps = _prep_in_maps(z, z_pos, z_dis, z_pos_dis, rand_idx)
    res = run_bass_kernel_spmd(nc, in_maps, core_ids=list(range(NCORES)))
    total = 0.0
    for c in range(NCORES):
        total += float(res.results[c]["out"].astype(np.float64).sum())
    loss = 0.5 * total / (B * NQ * NPOS)
    return np.float32(loss)
